# revision 42
# baseline (speedup 1.0000x reference)
"""Trainium2 Bass kernel for causal multi-head attention (B=4, T=2048, D=1024, H=16).

Sharding: 8 cores = 4 batches x 2 head-groups (8 heads each).
Per core pipeline (Tile framework, single SPMD program):
  phase 1(j): Q/K projections into transposed per-head-pair layout QT/KT [128=2*64, T],
           V projection into [t, 8*65] layout (65th col per head = ones, for rowsums)
  phase 2(j): per (q-range of 512, head-pair): causal flash attention in transposed
           layout: ST[k,q] = KT-slice^T @ QT-slice (row-packed pair of matmuls),
           PT = exp(ST) (ACT), causal triangle mask on diagonal 128-col strips (DVE),
           OT[hd+1, q] += [V|1]^T @ PT (bf16), normalize by approx-reciprocal rowsum.
  phase 3(j): output projection YT[dout, t] = Wo_gT^T @ OT, DMA'd straight from PSUM.
Phase 1(j+1) matmul chunks are emitted interleaved into phase 2(j) so the PE fills
its exp-wait gaps with projection work (phase 2 is ACT-bound; phases 1/3 PE-bound).
No collective: each core emits its partial YT [D, T]; the host adds the two partial
sums of each batch pair and adds the output bias.
"""

import numpy as np

B, T, D, H, HD = 4, 2048, 1024, 16, 64
NCORES = 8
NP = 4          # head pairs per core
NJ = 4          # q-ranges of 512
QW = 512
TB = T // 128   # 16

_CACHE = {}


def _build_nc():
    import concourse.mybir as mybir
    import concourse.tile as tile
    from concourse import bacc

    F32 = mybir.dt.float32
    BF16 = mybir.dt.bfloat16
    FP8 = mybir.dt.float8e4
    AF = mybir.ActivationFunctionType
    DR = mybir.MatmulPerfMode.DoubleRow

    nc = bacc.Bacc(None, target_bir_lowering=False)

    # Pin all activations to the one table holding Exp+Ln+Copy so the
    # act-table chooser can't thrash loads between the exp stream and the
    # exp(-ln(x)) reciprocal. Indices must match act_info.json, so other
    # tables are emptied rather than removed.
    import types as _types
    from concourse.hw_specs import get_activation_tables as _gat

    def _pinned_act_table_loads(self):
        import bass_rust as _bass_rust
        import concourse.mybir as _mybir
        has_activation = any(
            isinstance(i, _mybir.InstActivation)
            for b in self.main_func.blocks
            for i in b.instructions
        )
        if not has_activation:
            return
        tables = [
            (name, funcs if name == "natural_log_exp_and_others" else set())
            for name, funcs in _gat(self.m.arch).items()
        ]
        _bass_rust.insert_act_table_loads(self, tables)

    nc.insert_act_table_loads = _types.MethodType(_pinned_act_table_loads, nc)

    xt8_d = nc.declare_dram_parameter("xt8", [NJ, 128, 8 * QW], FP8, isOutput=False)
    xt_d = nc.declare_dram_parameter("xt", [NJ, 128, 8 * QW], BF16, isOutput=False)
    wq_d = nc.declare_dram_parameter("wq", [128, 8 * 512], FP8, isOutput=False)
    wk_d = nc.declare_dram_parameter("wk", [128, 8 * 512], FP8, isOutput=False)
    wv_d = nc.declare_dram_parameter("wv", [128, 8 * 512], BF16, isOutput=False)
    wo_d = nc.declare_dram_parameter("wo", [128, 4 * D], BF16, isOutput=False)
    mask_d = nc.declare_dram_parameter("mask", [128, 128], BF16, isOutput=False)
    yt_d = nc.declare_dram_parameter("yt", [D, T], F32, isOutput=True)

    with tile.TileContext(nc) as tc:
        with (
            tc.tile_pool(name="persist", bufs=1) as pers,
            tc.tile_pool(name="work", bufs=1) as work,
            tc.tile_pool(name="psum", bufs=1, space="PSUM") as psum,
        ):
            qt = pers.tile([128, NP, T], BF16)
            kt = pers.tile([128, NP, T], BF16)
            v = pers.tile([128, TB, 8 * 65], BF16)
            ot = pers.tile([128, NP, T], BF16)
            m0 = pers.tile([128, 128], BF16)
            wo = pers.tile([128, 4, D], BF16)
            wq = pers.tile([128, 8, 512], FP8)
            wk = pers.tile([128, 8, 512], FP8)
            wv = pers.tile([128, 8, 512], BF16)

            xs_tiles = {}
            xsb_tiles = {}

            def load_xs(j):
                t = work.tile([128, 8, QW], FP8, tag="xs", bufs=2)
                tb = work.tile([128, 8, QW], BF16, tag="xsb", bufs=2)
                nc.sync.dma_start(
                    out=t[:], in_=xt8_d[j].rearrange("p (c n) -> p c n", c=8)
                )
                nc.sync.dma_start(
                    out=tb[:], in_=xt_d[j].rearrange("p (c n) -> p c n", c=8)
                )
                xs_tiles[j] = t
                xsb_tiles[j] = tb

            # startup DMAs: weights are host-prearranged [128, ...] so each is
            # one fully contiguous transfer per partition
            nc.sync.dma_start(out=m0[:], in_=mask_d[:])
            for cp in range(4):
                cs = slice(cp * 1024, (cp + 1) * 1024)
                nc.sync.dma_start(
                    out=wq[:, 2 * cp:2 * cp + 2, :],
                    in_=wq_d[:, cs].rearrange("p (c n) -> p c n", c=2),
                )
                nc.sync.dma_start(
                    out=wk[:, 2 * cp:2 * cp + 2, :],
                    in_=wk_d[:, cs].rearrange("p (c n) -> p c n", c=2),
                )
            load_xs(0)
            for cp in range(4):
                cs = slice(cp * 1024, (cp + 1) * 1024)
                nc.sync.dma_start(
                    out=wv[:, 2 * cp:2 * cp + 2, :],
                    in_=wv_d[:, cs].rearrange("p (c n) -> p c n", c=2),
                )
            nc.sync.dma_start(out=wo[:], in_=wo_d.rearrange("p (c n) -> p c n", c=4))

            # weights are pre-scaled x128 on the host so they sit in e4m3's
            # normal range; the 1/128 is undone in the PSUM->SBUF copy below
            def emit_p1_qk(j, p, w_sb, dst):
                acc = psum.tile([128, QW], F32, tag="small", bufs=2)
                for cp in range(4):
                    nc.tensor.matmul(
                        acc[:],
                        w_sb[:, 2 * cp:2 * cp + 2, p * 128:(p + 1) * 128],
                        xs_tiles[j][:, 2 * cp:2 * cp + 2, :],
                        start=(cp == 0),
                        stop=(cp == 3),
                        perf_mode=DR,
                    )
                nc.vector.tensor_scalar_mul(
                    dst[:, p, j * QW:(j + 1) * QW], acc[:], 1.0 / 128.0
                )

            def emit_p1_v(j, sub):
                # V path stays bf16: early tokens average few keys, so v
                # quantization error doesn't wash out like q/k error does
                i = 4 * j + sub
                acc = psum.tile([128, QW], F32, tag="small", bufs=2)
                for c in range(8):
                    nc.tensor.matmul(
                        acc[:],
                        xsb_tiles[j][:, c, sub * 128:(sub + 1) * 128],
                        wv[:, c, :],
                        start=(c == 0),
                        stop=(c == 7),
                    )
                vblk = v[:, i, :].rearrange("p (h c) -> p h c", c=65)
                nc.vector.tensor_copy(
                    vblk[:, :, 0:64],
                    acc[:].rearrange("p (h c) -> p h c", c=64),
                )
                nc.gpsimd.memset(vblk[:, :, 64:65], 1.0)

            def phase1_chunks(j, v_early=False):
                ch = []
                for p in range(NP):
                    ch.append(lambda p=p: emit_p1_qk(j, p, wq, qt))
                    ch.append(lambda p=p: emit_p1_qk(j, p, wk, kt))
                    if v_early and p == 0:
                        for sub in range(4):
                            ch.append(lambda sub=sub: emit_p1_v(j, sub))
                if not v_early:
                    for sub in range(4):
                        ch.append(lambda sub=sub: emit_p1_v(j, sub))
                return ch

            # phase 1(0) up front, ordered so phase 2(0, p=0) unblocks early
            for chunk in phase1_chunks(0, v_early=True):
                chunk()

            def phase3_chunks(j):
                jrp = slice(j * QW, (j + 1) * QW)

                def emit_n(n):
                    yps = psum.tile([128, QW], F32, tag="small", bufs=2)
                    for c4 in range(4):
                        nc.tensor.matmul(
                            yps[:],
                            wo[:, c4, n * 128:(n + 1) * 128],
                            ot[:, c4, jrp],
                            start=(c4 == 0), stop=(c4 == 3),
                        )
                    ysb = work.tile([128, QW], F32, tag="ysb", bufs=3)
                    nc.vector.tensor_copy(ysb[:], yps[:])
                    nc.sync.dma_start(
                        out=yt_d[n * 128:(n + 1) * 128, jrp], in_=ysb[:]
                    )
                return [lambda n=n: emit_n(n) for n in range(8)]

            for j in range(NJ):
                jr = slice(j * QW, (j + 1) * QW)
                # PE filler work for this j's ACT-bound attention stream:
                # previous j's output projection + next j's projections
                p3 = phase3_chunks(j - 1) if j > 0 else []
                if j + 1 < NJ:
                    load_xs(j + 1)
                    p1 = phase1_chunks(j + 1)
                else:
                    p1 = []
                chunks = []
                for i in range(max(len(p3), len(p1))):
                    if i < len(p3):
                        chunks.append(p3[i])
                    if i < len(p1):
                        chunks.append(p1[i])
                nkb = 4 * j + 4
                slots = NP * nkb
                emitted = 0
                slot = 0

                # ---------------- phase 2(j) with phase 1(j+1) interleaved ----------
                for p in range(NP):
                    hA, hB = 2 * p, 2 * p + 1
                    o_A = psum.tile([65, QW], F32, tag="o", bufs=2)
                    o_B = psum.tile([65, QW], F32, tag="o", bufs=2)
                    for kb in range(nkb):
                        o = kb - 4 * j  # diagonal offset; < 0 means full block
                        lo = 128 * o if o > 0 else 0
                        st = psum.tile([128, 1024], F32, tag="st", bufs=2)
                        kcols = slice(kb * 128, (kb + 1) * 128)
                        qcols = slice(j * QW + lo, (j + 1) * QW)
                        nc.tensor.matmul(
                            st[:, lo:QW],
                            kt[0:64, p, kcols],
                            qt[0:64, p, qcols],
                            start=True, stop=True, tile_position=(0, 0),
                        )
                        nc.tensor.matmul(
                            st[:, QW + lo:2 * QW],
                            kt[64:128, p, kcols],
                            qt[64:128, p, qcols],
                            start=True, stop=True, tile_position=(64, 0),
                        )
                        pt = work.tile([128, 1024], BF16, tag="pt", bufs=3)
                        nc.scalar.activation(
                            pt[:].rearrange("p (h q) -> p h q", h=2)[:, :, lo:QW],
                            st[:].rearrange("p (h q) -> p h q", h=2)[:, :, lo:QW],
                            AF.Exp,
                        )
                        if o >= 0:
                            # only the leading 128-col strip of the valid range
                            # holds the causal triangle
                            nc.vector.tensor_mul(
                                pt[:, lo:lo + 128], pt[:, lo:lo + 128], m0[:]
                            )
                            nc.vector.tensor_mul(
                                pt[:, QW + lo:QW + lo + 128],
                                pt[:, QW + lo:QW + lo + 128],
                                m0[:],
                            )
                        nc.tensor.matmul(
                            o_A[:, lo:QW],
                            v[:, kb, hA * 65:(hA + 1) * 65],
                            pt[:, lo:QW],
                            start=(kb == 0), stop=(kb == nkb - 1),
                        )
                        nc.tensor.matmul(
                            o_B[:, lo:QW],
                            v[:, kb, hB * 65:(hB + 1) * 65],
                            pt[:, QW + lo:2 * QW],
                            start=(kb == 0), stop=(kb == nkb - 1),
                        )
                        slot += 1
                        while (
                            emitted < len(chunks)
                            and slot >= (emitted + 1) * slots // (len(chunks) + 3)
                        ):
                            chunks[emitted]()
                            emitted += 1
                    # stage o out of PSUM promptly so the o slots free for the
                    # next head-pair (keeps PE from stalling / HAM warm)
                    ocp = work.tile([65, 1024], F32, tag="ocp", bufs=5)
                    nc.vector.tensor_copy(ocp[:, 0:QW], o_A[:])
                    nc.vector.tensor_copy(ocp[:, QW:1024], o_B[:])
                    # normalize: 1/r = exp(-ln(r)) on ACT — Ln/Exp share the
                    # pinned act table with the main exp stream, no table load
                    lnr = work.tile([1, 1024], F32, tag="lnr", bufs=2)
                    nc.scalar.activation(lnr[:], ocp[64:65, :], AF.Ln)
                    rec = work.tile([1, 1024], F32, tag="rec", bufs=4)
                    nc.scalar.activation(rec[:], lnr[:], AF.Exp, scale=-1.0)
                    bc = work.tile([64, 1024], F32, tag="bc", bufs=3)
                    nc.gpsimd.partition_broadcast(bc[:, 0:QW], rec[:, 0:QW], channels=64)
                    nc.gpsimd.partition_broadcast(bc[:, QW:1024], rec[:, QW:1024], channels=64)
                    nc.vector.tensor_mul(ot[0:64, p, jr], ocp[0:64, 0:QW], bc[:, 0:QW])
                    nc.vector.tensor_mul(ot[64:128, p, jr], ocp[0:64, QW:1024], bc[:, QW:1024])
                while emitted < len(chunks):
                    chunks[emitted]()
                    emitted += 1

            # last j's output projection (the tail)
            for chunk in phase3_chunks(NJ - 1):
                chunk()

    nc.finalize()
    return nc


def _prep_inputs(x, Wq, Wk, Wv, Wo, bo):
    """Build the 8 per-core input maps (host-side layout prep only)."""
    import ml_dtypes

    scale = 1.0 / np.sqrt(np.float32(HD))
    kr = np.arange(128, dtype=np.float32)[:, None]
    qc = np.arange(128, dtype=np.float32)[None, :]
    m0 = (qc >= kr).astype(ml_dtypes.bfloat16)

    FP8 = ml_dtypes.float8_e4m3  # TRN FP8_EXP4-compatible for |x| <= 240

    def xarr(xb, dtype):  # [T, D] -> [NJ, 128, 8*512], one contiguous DMA per j
        xt = xb.T  # [D, T]
        out = np.stack(
            [
                xt[:, j * QW:(j + 1) * QW]
                .reshape(8, 128, QW).transpose(1, 0, 2).reshape(128, 8 * QW)
                for j in range(NJ)
            ]
        )
        return np.ascontiguousarray(out).astype(dtype)

    xt8s = [xarr(np.clip(x[b], -240, 240), FP8) for b in range(B)]
    xts = [xarr(x[b], ml_dtypes.bfloat16) for b in range(B)]
    in_maps = []
    for c in range(NCORES):
        b, g = c // 2, c % 2
        hs = slice(g * 8, (g + 1) * 8)
        # x128 prescale keeps the small weights inside e4m3's normal range;
        # the kernel multiplies the projection PSUM by 1/128 when casting out.
        # layouts are [128, c*...] so each weight loads as one contiguous DMA
        def warr(wt, dtype):  # [D, 512] -> [128, 8*512], row p = concat_c w[c*128+p]
            return np.ascontiguousarray(
                wt.reshape(8, 128, 512).transpose(1, 0, 2).reshape(128, 8 * 512)
            ).astype(dtype)

        wqc = warr(Wq[hs].reshape(512, D).T * (scale * 128), FP8)
        wkc = warr(Wk[hs].reshape(512, D).T * 128, FP8)
        wvc = warr(Wv[hs].reshape(512, D).T, ml_dtypes.bfloat16)
        woc = np.ascontiguousarray(
            Wo[:, g * 512:(g + 1) * 512].T.reshape(4, 128, D).transpose(1, 0, 2).reshape(128, 4 * D)
        ).astype(ml_dtypes.bfloat16)
        in_maps.append(
            {"xt8": xt8s[b], "xt": xts[b], "wq": wqc, "wk": wkc, "wv": wvc,
             "wo": woc, "mask": m0}
        )
    return in_maps


def _assemble(yts, bo):
    """Sum the per-core partial outputs of each batch pair, add bias."""
    y = np.empty((B, T, D), np.float32)
    for b in range(B):
        y[b] = (yts[2 * b] + yts[2 * b + 1]).T
    y += bo.astype(np.float32)[None, None, :]
    return y


def _run(inputs, trace=False, trace_cores=None):
    from concourse.bass_utils import run_bass_kernel_spmd

    if "nc" not in _CACHE:
        _CACHE["nc"] = _build_nc()
    nc = _CACHE["nc"]
    in_maps = _prep_inputs(
        inputs["x"], inputs["Wq"], inputs["Wk"], inputs["Wv"], inputs["Wo"], inputs["bo"]
    )
    r = run_bass_kernel_spmd(
        nc, in_maps, list(range(NCORES)), trace=trace, trace_cores=trace_cores
    )
    y = _assemble([r.results[c]["yt"] for c in range(NCORES)], inputs["bo"])
    return y, r


def kernel(**inputs):
    y, _ = _run(inputs, trace=False)
    return y


# revision 45
# speedup vs baseline: 1.2181x; 1.2181x over previous
"""Trainium2 Bass kernel for causal multi-head attention (B=4, T=2048, D=1024, H=16).

Sharding: 8 cores = 4 batches x 2 head-groups (8 heads each).
Per core pipeline (Tile framework, single SPMD program):
  phase 1(j): Q/K projections into transposed per-head-pair layout QT/KT [128=2*64, T],
           V projection into [t, 8*65] layout (65th col per head = ones, for rowsums)
  phase 2(j): per (q-range of 512, head-pair): causal flash attention in transposed
           layout: ST[k,q] = KT-slice^T @ QT-slice (row-packed pair of matmuls),
           PT = exp(ST) (ACT), causal triangle mask on diagonal 128-col strips (DVE),
           OT[hd+1, q] += [V|1]^T @ PT (bf16), normalize by approx-reciprocal rowsum.
  phase 3(j): output projection YT[dout, t] = Wo_gT^T @ OT, DMA'd straight from PSUM.
Phase 1(j+1) matmul chunks are emitted interleaved into phase 2(j) so the PE fills
its exp-wait gaps with projection work (phase 2 is ACT-bound; phases 1/3 PE-bound).
No collective: each core emits its partial YT [D, T]; the host adds the two partial
sums of each batch pair and adds the output bias.
"""

import numpy as np

B, T, D, H, HD = 4, 2048, 1024, 16, 64
NCORES = 8
NP = 4          # head pairs per core
NJ = 4          # q-ranges of 512
QW = 512
TB = T // 128   # 16

_CACHE = {}


def _build_nc():
    import concourse.mybir as mybir
    import concourse.tile as tile
    from concourse import bacc

    F32 = mybir.dt.float32
    BF16 = mybir.dt.bfloat16
    FP8 = mybir.dt.float8e4
    AF = mybir.ActivationFunctionType
    DR = mybir.MatmulPerfMode.DoubleRow

    nc = bacc.Bacc(None, target_bir_lowering=False)

    # Pin all activations to the one table holding Exp+Ln+Copy so the
    # act-table chooser can't thrash loads between the exp stream and the
    # exp(-ln(x)) reciprocal. Indices must match act_info.json, so other
    # tables are emptied rather than removed.
    import types as _types
    from concourse.hw_specs import get_activation_tables as _gat

    def _pinned_act_table_loads(self):
        import bass_rust as _bass_rust
        import concourse.mybir as _mybir
        has_activation = any(
            isinstance(i, _mybir.InstActivation)
            for b in self.main_func.blocks
            for i in b.instructions
        )
        if not has_activation:
            return
        tables = [
            (name, funcs if name == "natural_log_exp_and_others" else set())
            for name, funcs in _gat(self.m.arch).items()
        ]
        _bass_rust.insert_act_table_loads(self, tables)

    nc.insert_act_table_loads = _types.MethodType(_pinned_act_table_loads, nc)

    xt8_d = nc.declare_dram_parameter("xt8", [NJ, 128, 8 * QW], FP8, isOutput=False)
    xt_d = nc.declare_dram_parameter("xt", [NJ, 128, 8 * QW], BF16, isOutput=False)
    wq_d = nc.declare_dram_parameter("wq", [128, 8 * 512], FP8, isOutput=False)
    wk_d = nc.declare_dram_parameter("wk", [128, 8 * 512], FP8, isOutput=False)
    wv_d = nc.declare_dram_parameter("wv", [128, 8 * 512], BF16, isOutput=False)
    wo_d = nc.declare_dram_parameter("wo", [128, 4 * D], BF16, isOutput=False)
    mask_d = nc.declare_dram_parameter("mask", [128, 128], BF16, isOutput=False)
    yt_d = nc.declare_dram_parameter("yt", [D, T], F32, isOutput=True)

    with tile.TileContext(nc) as tc:
        with (
            tc.tile_pool(name="persist", bufs=1) as pers,
            tc.tile_pool(name="work", bufs=1) as work,
            tc.tile_pool(name="psum", bufs=1, space="PSUM") as psum,
        ):
            qt = pers.tile([128, NP, T], BF16)
            kt = pers.tile([128, NP, T], BF16)
            v = pers.tile([128, TB, 8 * 65], BF16)
            ot = pers.tile([128, NP, T], BF16)
            m0 = pers.tile([128, 128], BF16)
            wo = pers.tile([128, 4, D], BF16)
            wq = pers.tile([128, 8, 512], FP8)
            wk = pers.tile([128, 8, 512], FP8)
            wv = pers.tile([128, 8, 512], BF16)

            xs_tiles = {}
            xsb_tiles = {}

            def load_xs(j):
                t = work.tile([128, 8, QW], FP8, tag="xs", bufs=2)
                tb = work.tile([128, 8, QW], BF16, tag="xsb", bufs=2)
                nc.sync.dma_start(
                    out=t[:], in_=xt8_d[j].rearrange("p (c n) -> p c n", c=8)
                )
                nc.sync.dma_start(
                    out=tb[:], in_=xt_d[j].rearrange("p (c n) -> p c n", c=8)
                )
                xs_tiles[j] = t
                xsb_tiles[j] = tb

            # startup DMAs: weights are host-prearranged [128, ...] so each is
            # one fully contiguous transfer per partition
            nc.sync.dma_start(out=m0[:], in_=mask_d[:])
            nc.sync.dma_start(out=wq[:], in_=wq_d.rearrange("p (c n) -> p c n", c=8))
            nc.sync.dma_start(out=wk[:], in_=wk_d.rearrange("p (c n) -> p c n", c=8))
            load_xs(0)
            nc.sync.dma_start(out=wv[:], in_=wv_d.rearrange("p (c n) -> p c n", c=8))
            nc.sync.dma_start(out=wo[:], in_=wo_d.rearrange("p (c n) -> p c n", c=4))

            # weights are pre-scaled x128 on the host so they sit in e4m3's
            # normal range; the 1/128 is undone in the PSUM->SBUF copy below
            def emit_p1_qk(j, p, w_sb, dst):
                acc = psum.tile([128, QW], F32, tag="small", bufs=2)
                for cp in range(4):
                    nc.tensor.matmul(
                        acc[:],
                        w_sb[:, 2 * cp:2 * cp + 2, p * 128:(p + 1) * 128],
                        xs_tiles[j][:, 2 * cp:2 * cp + 2, :],
                        start=(cp == 0),
                        stop=(cp == 3),
                        perf_mode=DR,
                    )
                nc.vector.tensor_scalar_mul(
                    dst[:, p, j * QW:(j + 1) * QW], acc[:], 1.0 / 128.0
                )

            def emit_p1_v(j, sub):
                # V path stays bf16: early tokens average few keys, so v
                # quantization error doesn't wash out like q/k error does
                i = 4 * j + sub
                acc = psum.tile([128, QW], F32, tag="small", bufs=2)
                for c in range(8):
                    nc.tensor.matmul(
                        acc[:],
                        xsb_tiles[j][:, c, sub * 128:(sub + 1) * 128],
                        wv[:, c, :],
                        start=(c == 0),
                        stop=(c == 7),
                    )
                vblk = v[:, i, :].rearrange("p (h c) -> p h c", c=65)
                nc.vector.tensor_copy(
                    vblk[:, :, 0:64],
                    acc[:].rearrange("p (h c) -> p h c", c=64),
                )
                nc.gpsimd.memset(vblk[:, :, 64:65], 1.0)

            def phase1_chunks(j, v_early=False):
                ch = []
                for p in range(NP):
                    ch.append(lambda p=p: emit_p1_qk(j, p, wq, qt))
                    ch.append(lambda p=p: emit_p1_qk(j, p, wk, kt))
                    if v_early and p == 0:
                        for sub in range(4):
                            ch.append(lambda sub=sub: emit_p1_v(j, sub))
                if not v_early:
                    for sub in range(4):
                        ch.append(lambda sub=sub: emit_p1_v(j, sub))
                return ch

            # phase 1(0) up front, ordered so phase 2(0, p=0) unblocks early
            for chunk in phase1_chunks(0, v_early=True):
                chunk()

            def phase3_chunks(j):
                jrp = slice(j * QW, (j + 1) * QW)

                def emit_n(n):
                    yps = psum.tile([128, QW], F32, tag="small", bufs=2)
                    for c4 in range(4):
                        nc.tensor.matmul(
                            yps[:],
                            wo[:, c4, n * 128:(n + 1) * 128],
                            ot[:, c4, jrp],
                            start=(c4 == 0), stop=(c4 == 3),
                        )
                    ysb = work.tile([128, QW], F32, tag="ysb", bufs=3)
                    nc.vector.tensor_copy(ysb[:], yps[:])
                    nc.sync.dma_start(
                        out=yt_d[n * 128:(n + 1) * 128, jrp], in_=ysb[:]
                    )
                return [lambda n=n: emit_n(n) for n in range(8)]

            for j in range(NJ):
                jr = slice(j * QW, (j + 1) * QW)
                # PE filler work for this j's ACT-bound attention stream:
                # previous j's output projection + next j's projections
                p3 = phase3_chunks(j - 1) if j > 0 else []
                if j + 1 < NJ:
                    load_xs(j + 1)
                    p1 = phase1_chunks(j + 1)
                else:
                    p1 = []
                chunks = []
                for i in range(max(len(p3), len(p1))):
                    if i < len(p3):
                        chunks.append(p3[i])
                    if i < len(p1):
                        chunks.append(p1[i])
                nkb = 4 * j + 4
                slots = NP * nkb
                emitted = 0
                slot = 0
                ocps = []

                # ---------------- phase 2(j) with phase 1(j+1) interleaved ----------
                for p in range(NP):
                    hA, hB = 2 * p, 2 * p + 1
                    o_A = psum.tile([65, QW], F32, tag="o", bufs=2)
                    o_B = psum.tile([65, QW], F32, tag="o", bufs=2)
                    for kb in range(nkb):
                        o = kb - 4 * j  # diagonal offset; < 0 means full block
                        lo = 128 * o if o > 0 else 0
                        st = psum.tile([128, 1024], F32, tag="st", bufs=2)
                        kcols = slice(kb * 128, (kb + 1) * 128)
                        qcols = slice(j * QW + lo, (j + 1) * QW)
                        nc.tensor.matmul(
                            st[:, lo:QW],
                            kt[0:64, p, kcols],
                            qt[0:64, p, qcols],
                            start=True, stop=True, tile_position=(0, 0),
                        )
                        nc.tensor.matmul(
                            st[:, QW + lo:2 * QW],
                            kt[64:128, p, kcols],
                            qt[64:128, p, qcols],
                            start=True, stop=True, tile_position=(64, 0),
                        )
                        pt = work.tile([128, 1024], BF16, tag="pt", bufs=3)
                        nc.scalar.activation(
                            pt[:].rearrange("p (h q) -> p h q", h=2)[:, :, lo:QW],
                            st[:].rearrange("p (h q) -> p h q", h=2)[:, :, lo:QW],
                            AF.Exp,
                        )
                        if o >= 0:
                            # only the leading 128-col strip of the valid range
                            # holds the causal triangle
                            nc.vector.tensor_mul(
                                pt[:, lo:lo + 128], pt[:, lo:lo + 128], m0[:]
                            )
                            nc.vector.tensor_mul(
                                pt[:, QW + lo:QW + lo + 128],
                                pt[:, QW + lo:QW + lo + 128],
                                m0[:],
                            )
                        nc.tensor.matmul(
                            o_A[:, lo:QW],
                            v[:, kb, hA * 65:(hA + 1) * 65],
                            pt[:, lo:QW],
                            start=(kb == 0), stop=(kb == nkb - 1),
                        )
                        nc.tensor.matmul(
                            o_B[:, lo:QW],
                            v[:, kb, hB * 65:(hB + 1) * 65],
                            pt[:, QW + lo:2 * QW],
                            start=(kb == 0), stop=(kb == nkb - 1),
                        )
                        slot += 1
                        while (
                            emitted < len(chunks)
                            and slot >= (emitted + 1) * slots // (len(chunks) + 3)
                        ):
                            chunks[emitted]()
                            emitted += 1
                    # stage o out of PSUM promptly so the o slots free for the
                    # next head-pair (keeps PE from stalling / HAM warm)
                    ocp = work.tile([65, 1024], F32, tag="ocp", bufs=5)
                    nc.vector.tensor_copy(ocp[:, 0:QW], o_A[:])
                    nc.vector.tensor_copy(ocp[:, QW:1024], o_B[:])
                    ocps.append(ocp)
                # batched normalize for the whole j, emitted after all of the
                # j's exps: per-p emission would head-of-line-block the ACT
                # FIFO (ln waits on a DVE copy while kb-exps queue behind it).
                # 1/r = exp(-ln(r)): Ln/Exp share the pinned act table.
                recs = []
                for p in range(NP):
                    lnr = work.tile([1, 1024], F32, tag="lnr", bufs=2)
                    nc.scalar.activation(lnr[:], ocps[p][64:65, :], AF.Ln)
                    rec = work.tile([1, 1024], F32, tag="rec", bufs=4)
                    nc.scalar.activation(rec[:], lnr[:], AF.Exp, scale=-1.0)
                    recs.append(rec)
                for p in range(NP):
                    bc = work.tile([64, 1024], F32, tag="bc", bufs=3)
                    nc.gpsimd.partition_broadcast(bc[:, 0:QW], recs[p][:, 0:QW], channels=64)
                    nc.gpsimd.partition_broadcast(bc[:, QW:1024], recs[p][:, QW:1024], channels=64)
                    nc.vector.tensor_mul(ot[0:64, p, jr], ocps[p][0:64, 0:QW], bc[:, 0:QW])
                    nc.vector.tensor_mul(ot[64:128, p, jr], ocps[p][0:64, QW:1024], bc[:, QW:1024])
                ocps.clear()
                while emitted < len(chunks):
                    chunks[emitted]()
                    emitted += 1

            # last j's output projection (the tail)
            for chunk in phase3_chunks(NJ - 1):
                chunk()

    nc.finalize()
    return nc


def _prep_inputs(x, Wq, Wk, Wv, Wo, bo):
    """Build the 8 per-core input maps (host-side layout prep only)."""
    import ml_dtypes

    scale = 1.0 / np.sqrt(np.float32(HD))
    kr = np.arange(128, dtype=np.float32)[:, None]
    qc = np.arange(128, dtype=np.float32)[None, :]
    m0 = (qc >= kr).astype(ml_dtypes.bfloat16)

    FP8 = ml_dtypes.float8_e4m3  # TRN FP8_EXP4-compatible for |x| <= 240

    def xarr(xb, dtype):  # [T, D] -> [NJ, 128, 8*512], one contiguous DMA per j
        xt = xb.T  # [D, T]
        out = np.stack(
            [
                xt[:, j * QW:(j + 1) * QW]
                .reshape(8, 128, QW).transpose(1, 0, 2).reshape(128, 8 * QW)
                for j in range(NJ)
            ]
        )
        return np.ascontiguousarray(out).astype(dtype)

    xt8s = [xarr(np.clip(x[b], -240, 240), FP8) for b in range(B)]
    xts = [xarr(x[b], ml_dtypes.bfloat16) for b in range(B)]
    in_maps = []
    for c in range(NCORES):
        b, g = c // 2, c % 2
        hs = slice(g * 8, (g + 1) * 8)
        # x128 prescale keeps the small weights inside e4m3's normal range;
        # the kernel multiplies the projection PSUM by 1/128 when casting out.
        # layouts are [128, c*...] so each weight loads as one contiguous DMA
        def warr(wt, dtype):  # [D, 512] -> [128, 8*512], row p = concat_c w[c*128+p]
            return np.ascontiguousarray(
                wt.reshape(8, 128, 512).transpose(1, 0, 2).reshape(128, 8 * 512)
            ).astype(dtype)

        wqc = warr(Wq[hs].reshape(512, D).T * (scale * 128), FP8)
        wkc = warr(Wk[hs].reshape(512, D).T * 128, FP8)
        wvc = warr(Wv[hs].reshape(512, D).T, ml_dtypes.bfloat16)
        woc = np.ascontiguousarray(
            Wo[:, g * 512:(g + 1) * 512].T.reshape(4, 128, D).transpose(1, 0, 2).reshape(128, 4 * D)
        ).astype(ml_dtypes.bfloat16)
        in_maps.append(
            {"xt8": xt8s[b], "xt": xts[b], "wq": wqc, "wk": wkc, "wv": wvc,
             "wo": woc, "mask": m0}
        )
    return in_maps


def _assemble(yts, bo):
    """Sum the per-core partial outputs of each batch pair, add bias."""
    y = np.empty((B, T, D), np.float32)
    for b in range(B):
        y[b] = (yts[2 * b] + yts[2 * b + 1]).T
    y += bo.astype(np.float32)[None, None, :]
    return y


def _run(inputs, trace=False, trace_cores=None):
    from concourse.bass_utils import run_bass_kernel_spmd

    if "nc" not in _CACHE:
        _CACHE["nc"] = _build_nc()
    nc = _CACHE["nc"]
    in_maps = _prep_inputs(
        inputs["x"], inputs["Wq"], inputs["Wk"], inputs["Wv"], inputs["Wo"], inputs["bo"]
    )
    r = run_bass_kernel_spmd(
        nc, in_maps, list(range(NCORES)), trace=trace, trace_cores=trace_cores
    )
    y = _assemble([r.results[c]["yt"] for c in range(NCORES)], inputs["bo"])
    return y, r


def kernel(**inputs):
    y, _ = _run(inputs, trace=False)
    return y
